# revision 1
# baseline (speedup 1.0000x reference)
"""Trainium2 Bass kernel for nn_Encoder (pre-norm transformer block, LN over
sequence axis) distributed over 8 NeuronCores.

Sharding:
  - LN1/LN2 channel-sharded (C/8 = 128 channels per core, [chan, T] layout)
  - attention head-sharded (2 heads x 2 batches per core), scores computed
    transposed (S^T = k q^T) so softmax sums run through the PE via a
    ones-column appended to V, and no P transpose is needed
  - per-batch AllGather(h^T), AllGather(attn^T) in bf16 (0.5 MB/rank each),
    issued as soon as each batch is ready so they overlap compute
  - Wo column-sharded (rhs streamed from the gathered attn^T), LN2 on the
    channel slice, then AllToAll(h2^T) bf16 + AllToAll(y^T) fp32 switch from
    channel-sharded to token-sharded; FFN token-sharded with full W1/W2
  - output assembled on host from per-core token slices
"""

import numpy as np
import ml_dtypes
from contextlib import ExitStack

from concourse import bacc, bass_utils
import concourse.bass as bass
import concourse.tile as tile
import concourse.mybir as mybir
from concourse.masks import make_identity

FP32 = mybir.dt.float32
BF16 = mybir.dt.bfloat16
AF = mybir.ActivationFunctionType
ALU = mybir.AluOpType
AX = mybir.AxisListType

B, T, C, H, HS = 2, 2048, 1024, 16, 64
NCORE, P = 8, 128
TN = B * T            # 4096 flat tokens
TOK = TN // NCORE     # 512 tokens per core
F = 4 * C             # 4096
KK = C // P           # 8 k-tiles over C
EPS = 1e-5
RG = [list(range(NCORE))]

_cache = {}


def _ln_stats(nc, pool, xsrc, g_sb, be_sb, n):
    """Per-partition LN coefficients over the free axis of xsrc [P, n].
    Returns (A, Bv) with h = x*A + Bv. Unbiased var, eps outside sqrt."""
    s1 = pool.tile([P, 1], FP32, tag="s1")
    s2 = pool.tile([P, 1], FP32, tag="s2")
    scr = pool.tile([P, n], FP32, tag="scr")
    nc.vector.reduce_sum(s1[:], xsrc, axis=AX.X)
    nc.vector.scalar_tensor_tensor(
        out=scr[:], in0=xsrc, scalar=1.0, in1=xsrc,
        op0=ALU.mult, op1=ALU.mult, accum_out=s2[:])
    mean = pool.tile([P, 1], FP32, tag="mean")
    nc.vector.tensor_scalar_mul(mean[:], s1[:], 1.0 / n)
    ss = pool.tile([P, 1], FP32, tag="ss")
    nc.vector.tensor_mul(ss[:], s1[:], s1[:])
    var = pool.tile([P, 1], FP32, tag="var")
    nc.vector.scalar_tensor_tensor(
        out=var[:], in0=ss[:], scalar=-1.0 / n, in1=s2[:],
        op0=ALU.mult, op1=ALU.add)
    nc.vector.tensor_scalar_mul(var[:], var[:], 1.0 / (n - 1))
    den = pool.tile([P, 1], FP32, tag="den")
    nc.scalar.sqrt(den[:], var[:])
    nc.vector.tensor_scalar_add(den[:], den[:], EPS)
    rden = pool.tile([P, 1], FP32, tag="rden")
    nc.vector.reciprocal(rden[:], den[:])
    A = pool.tile([P, 1], FP32, tag="A")
    nc.vector.tensor_mul(A[:], g_sb, rden[:])
    mA = pool.tile([P, 1], FP32, tag="mA")
    nc.vector.tensor_scalar_mul(mA[:], mean[:], A[:])
    Bv = pool.tile([P, 1], FP32, tag="Bv")
    nc.vector.tensor_sub(Bv[:], be_sb, mA[:])
    return A, Bv


def build():
    nc = bacc.Bacc("TRN2", target_bir_lowering=False, debug=False,
                   num_devices=NCORE)

    def EIN(name, shape, dtype):
        return nc.dram_tensor(name, shape, dtype, kind="ExternalInput")

    x_c = EIN("x_c", [TN, P], FP32)        # x[:, :, ci]  (flat tokens, my chans)
    wq = EIN("wq", [P, KK, P], BF16)       # Wq cat(2 heads) tiled [p, kk, m]
    wk = EIN("wk", [P, KK, P], BF16)
    wv = EIN("wv", [P, KK, P], BF16)
    woc = EIN("woc", [P, KK, P], BF16)     # Wo[:, ci] tiled
    w1t = EIN("w1t", [F // P, P, KK, P], BF16)  # [32, p, kk, mc]
    w2t = EIN("w2t", [P, F // P, C], BF16)      # [p, q, n]
    bqc = EIN("bqc", [P, 1], FP32)
    bkc = EIN("bkc", [P, 1], FP32)
    boc = EIN("boc", [P, 1], FP32)
    b1t = EIN("b1t", [P, F // P], FP32)    # [p, m]
    b2r = EIN("b2r", [1, C], FP32)         # b2 row (added via ones-row matmul)
    g1 = EIN("g1", [P, 1], FP32)
    be1 = EIN("be1", [P, 1], FP32)
    g2 = EIN("g2", [P, 1], FP32)
    be2 = EIN("be2", [P, 1], FP32)
    out = nc.dram_tensor("out", [TOK, C], FP32, kind="ExternalOutput")

    with tile.TileContext(nc) as tc, ExitStack() as ctx:
        const = ctx.enter_context(tc.tile_pool(name="const", bufs=1))
        dram = ctx.enter_context(tc.tile_pool(name="dram", bufs=1, space="DRAM"))
        persist = ctx.enter_context(tc.tile_pool(name="acts", bufs=1))

        ident = const.tile([P, P], FP32)
        make_identity(nc, ident)
        ones1 = const.tile([1, P], FP32)
        nc.vector.memset(ones1[:], 1.0)

        def ldconst(t, shape, dt=FP32):
            s = const.tile(shape, dt, name=t.name + "_sb")
            nc.sync.dma_start(s[:], t.ap())
            return s

        bq_sb = ldconst(bqc, [P, 1])
        bk_sb = ldconst(bkc, [P, 1])
        bo_sb = ldconst(boc, [P, 1])
        b1_sb = ldconst(b1t, [P, F // P])
        b2_sb = ldconst(b2r, [1, C])
        g1_sb = ldconst(g1, [P, 1])
        be1_sb = ldconst(be1, [P, 1])
        g2_sb = ldconst(g2, [P, 1])
        be2_sb = ldconst(be2, [P, 1])
        wq_sb = ldconst(wq, [P, KK, P], BF16)
        wk_sb = ldconst(wk, [P, KK, P], BF16)
        wv_sb = ldconst(wv, [P, KK, P], BF16)
        woc_sb = ldconst(woc, [P, KK, P], BF16)

        # activations that live across phases 1-3
        xT = persist.tile([P, B, T], FP32)
        attnT_loc = persist.tile([P, TN], BF16)
        h2T_loc = persist.tile([P, TN], BF16)
        yT = persist.tile([P, B, T], FP32)
        hT_loc = persist.tile([P, B, T], BF16)
        qT_sb = persist.tile([P, B, T], BF16)
        kT_sb = persist.tile([P, B, T], BF16)
        vaug = persist.tile([P, B, 2, T // P, 65], BF16)

        # DRAM comm tiles (per-batch AGs so they overlap compute)
        agh_in = [dram.tile([P, T], BF16, name=f"agh_in{b}") for b in range(B)]
        agh_out = [dram.tile([C, T], BF16, addr_space="Shared",
                             name=f"agh_out{b}") for b in range(B)]
        aga_in = [dram.tile([P, T], BF16, name=f"aga_in{b}") for b in range(B)]
        aga_out = [dram.tile([C, T], BF16, addr_space="Shared",
                             name=f"aga_out{b}") for b in range(B)]
        a2h_in = dram.tile([NCORE, P, TOK], BF16)
        a2h_out = dram.tile([NCORE, P, TOK], BF16)
        a2y_in = dram.tile([NCORE, P, TOK], FP32)
        a2y_out = dram.tile([NCORE, P, TOK], FP32)

        # ---------------- Phase 1: transpose x slice + LN1 (per batch) ------
        with tc.tile_pool(name="ph1", bufs=4) as ph1, \
             tc.tile_pool(name="ph1p", bufs=4, space="PSUM") as ph1p, \
             tc.tile_pool(name="stats", bufs=2) as stats:
            for b in range(B):
                for tt in range(T // P):
                    xc_t = ph1.tile([P, P], FP32, tag="xc")
                    nc.sync.dma_start(
                        xc_t[:], x_c.ap()[b * T + tt * P: b * T + (tt + 1) * P, :])
                    tp = ph1p.tile([P, P], FP32, tag="tp")
                    nc.tensor.transpose(tp[:], xc_t[:], ident[:])
                    nc.vector.tensor_copy(xT[:, b, tt * P:(tt + 1) * P], tp[:])
                A, Bv = _ln_stats(nc, stats, xT[:, b, :], g1_sb[:], be1_sb[:], T)
                nc.vector.tensor_scalar(
                    out=hT_loc[:, b, :], in0=xT[:, b, :],
                    scalar1=A[:], scalar2=Bv[:], op0=ALU.mult, op1=ALU.add)
                nc.sync.dma_start(agh_in[b][:], hT_loc[:, b, :])
                nc.gpsimd.collective_compute(
                    "AllGather", ALU.bypass, replica_groups=RG,
                    ins=[agh_in[b].opt()], outs=[agh_out[b].opt()])

        # ---------------- Phase 2a: QKV ----------------
        nc.vector.memset(vaug[:, :, :, :, 64], 1.0)
        with tc.tile_pool(name="hst", bufs=1) as hst, \
             tc.tile_pool(name="qkp", bufs=4, space="PSUM") as qkp:
            hT_st = hst.tile([P, KK, B, T], BF16)   # 64KB/part, freed post-QKV
            for kk in range(KK):
                for b in range(B):
                    nc.sync.dma_start(
                        hT_st[:, kk, b, :],
                        agh_out[b].rearrange("(kk p) n -> p kk n", p=P)[:, kk, :])
            for b in range(B):
                for w_sb, bias_sb, dst in ((wq_sb, bq_sb, qT_sb),
                                           (wk_sb, bk_sb, kT_sb)):
                    for j in range(T // 512):
                        ps = qkp.tile([P, 512], FP32, tag="mm")
                        for kk in range(KK):
                            nc.tensor.matmul(
                                ps[:], lhsT=w_sb[:, kk, :],
                                rhs=hT_st[:, kk, b, j * 512:(j + 1) * 512],
                                start=(kk == 0), stop=(kk == KK - 1))
                        nc.vector.tensor_scalar_add(
                            dst[:, b, j * 512:(j + 1) * 512], ps[:], bias_sb[:])
                for tt in range(T // P):
                    vps_full = qkp.tile([P, 512], FP32, tag="mm", name="vps")
                    vps = vps_full[:, 0:P]
                    for kk in range(KK):
                        nc.tensor.matmul(
                            vps, lhsT=hT_st[:, kk, b, tt * P:(tt + 1) * P],
                            rhs=wv_sb[:, kk, :],
                            start=(kk == 0), stop=(kk == KK - 1))
                    for hd in range(2):
                        nc.vector.tensor_copy(
                            vaug[:, b, hd, tt, 0:64],
                            vps[:, hd * 64:(hd + 1) * 64])

        # ---------------- Phase 2b: attention ----------------
        with tc.tile_pool(name="ph2l", bufs=6) as ph2l, \
             tc.tile_pool(name="sp", bufs=2, space="PSUM") as sp, \
             tc.tile_pool(name="attp", bufs=3, space="PSUM") as attp:
            for b in range(B):
                for hd in range(2):
                    att_h = [attp.tile([65, T // 2], FP32, tag="att",
                                       name=f"att{b}{hd}{jh}") for jh in range(2)]
                    for k in range(T // P):
                        p_tiles = []
                        for j in range(T // 512):
                            s_ps = sp.tile([P, 512], FP32, tag="s")
                            nc.tensor.matmul(
                                s_ps[:],
                                lhsT=kT_sb[hd * 64:(hd + 1) * 64, b, k * P:(k + 1) * P],
                                rhs=qT_sb[hd * 64:(hd + 1) * 64, b, j * 512:(j + 1) * 512],
                                start=True, stop=True)
                            p_sb = ph2l.tile([P, 512], BF16, tag="p",
                                             name=f"p{j}")
                            nc.scalar.activation(p_sb[:], s_ps[:], AF.Exp,
                                                 scale=float(HS) ** -0.5)
                            p_tiles.append(p_sb)
                        for j in range(T // 512):
                            nc.tensor.matmul(
                                att_h[j // 2][:, (j % 2) * 512:(j % 2 + 1) * 512],
                                lhsT=vaug[:, b, hd, k, :], rhs=p_tiles[j][:],
                                start=(k == 0), stop=(k == T // P - 1))
                    for jh in range(2):
                        rden = ph2l.tile([1, T // 2], FP32, tag="rden")
                        nc.vector.reciprocal(rden[:], att_h[jh][64:65, :])
                        for jq in range(2):
                            rdps_f = sp.tile([P, 512], FP32, tag="s", name="rdps")
                            rdps = rdps_f[0:64, :]
                            nc.tensor.matmul(
                                rdps, lhsT=ones1[:, 0:64],
                                rhs=rden[:, jq * 512:(jq + 1) * 512],
                                start=True, stop=True)
                            rd_sb = ph2l.tile([64, 512], FP32, tag="rd_sb")
                            nc.vector.tensor_copy(rd_sb[:], rdps)
                            nc.vector.tensor_mul(
                                attnT_loc[hd * 64:(hd + 1) * 64,
                                          b * T + jh * 1024 + jq * 512:
                                          b * T + jh * 1024 + (jq + 1) * 512],
                                att_h[jh][0:64, jq * 512:(jq + 1) * 512], rd_sb[:])
                nc.sync.dma_start(aga_in[b][:], attnT_loc[:, b * T:(b + 1) * T])
                nc.gpsimd.collective_compute(
                    "AllGather", ALU.bypass, replica_groups=RG,
                    ins=[aga_in[b].opt()], outs=[aga_out[b].opt()])

        # ---------------- Phase 3: Wo col-shard (streamed rhs) + LN2 --------
        with tc.tile_pool(name="ph3", bufs=16) as ph3, \
             tc.tile_pool(name="ph3p", bufs=4, space="PSUM") as ph3p, \
             tc.tile_pool(name="stats3", bufs=2) as stats3:
            for b in range(B):
                for j in range(T // 512):
                    yps = ph3p.tile([P, 512], FP32, tag="y")
                    for kk in range(KK):
                        a_t = ph3.tile([P, 512], BF16, tag="a_t")
                        src_v = aga_out[b].rearrange("(kk p) n -> p kk n", p=P)
                        nc.sync.dma_start(
                            a_t[:, 0:256],
                            src_v[:, kk, j * 512: j * 512 + 256])
                        nc.gpsimd.dma_start(
                            a_t[:, 256:512],
                            src_v[:, kk, j * 512 + 256:(j + 1) * 512])
                        nc.tensor.matmul(
                            yps[:], lhsT=woc_sb[:, kk, :], rhs=a_t[:],
                            start=(kk == 0), stop=(kk == KK - 1))
                    nc.vector.scalar_tensor_tensor(
                        out=yT[:, b, j * 512:(j + 1) * 512], in0=yps[:],
                        scalar=bo_sb[:], in1=xT[:, b, j * 512:(j + 1) * 512],
                        op0=ALU.add, op1=ALU.add)
                A2, Bv2 = _ln_stats(nc, stats3, yT[:, b, :], g2_sb[:], be2_sb[:], T)
                nc.vector.tensor_scalar(
                    out=h2T_loc[:, b * T:(b + 1) * T], in0=yT[:, b, :],
                    scalar1=A2[:], scalar2=Bv2[:], op0=ALU.mult, op1=ALU.add)

        for j in range(NCORE):
            nc.sync.dma_start(a2h_in[j], h2T_loc[:, j * TOK:(j + 1) * TOK])
        nc.gpsimd.collective_compute(
            "AllToAll", ALU.bypass, replica_groups=RG,
            ins=[a2h_in.opt()], outs=[a2h_out.opt()])
        for j in range(NCORE):
            nc.sync.dma_start(
                a2y_in[j], yT.rearrange("p b t -> p (b t)")[:, j * TOK:(j + 1) * TOK])
        nc.gpsimd.collective_compute(
            "AllToAll", ALU.bypass, replica_groups=RG,
            ins=[a2y_in.opt()], outs=[a2y_out.opt()])

        # ---------------- Phase 4: FFN token-sharded ----------------
        with tc.tile_pool(name="ph4", bufs=1) as ph4, \
             tc.tile_pool(name="ph4l", bufs=4) as ph4l, \
             tc.tile_pool(name="ph4o", bufs=2) as ph4o:
            h2tok = ph4.tile([P, KK, TOK], BF16)
            engs = (nc.sync, nc.gpsimd, nc.sync, nc.gpsimd)
            for kk in range(KK):
                engs[kk % 4].dma_start(h2tok[:, kk, :], a2h_out[kk])
            ytok = ph4.tile([P, TOK // P, C], FP32)   # 16KB/part
            uT = ph4.tile([P, F // P, TOK], BF16)     # 32KB/part
            with tc.tile_pool(name="up", bufs=4, space="PSUM") as up:
                # y blocks: stage, PE-transpose to token-major [tok, chan]
                for kk in range(KK):
                    yb = ph4l.tile([P, TOK], FP32, tag="yb")
                    engs[kk % 4].dma_start(yb[:], a2y_out[kk])
                    for tt in range(TOK // P):
                        ytp = up.tile([P, P], FP32, tag="u", name="ytp")
                        nc.tensor.transpose(ytp[:], yb[:, tt * P:(tt + 1) * P],
                                            ident[:])
                        nc.vector.tensor_copy(ytok[:, tt, kk * P:(kk + 1) * P],
                                              ytp[:])
                for m in range(F // P):
                    w1_sl = ph4l.tile([P, KK, P], BF16, tag="w1", bufs=6)
                    nc.sync.dma_start(w1_sl[:, 0:KK // 2, :], w1t.ap()[m][:, 0:KK // 2, :])
                    nc.gpsimd.dma_start(w1_sl[:, KK // 2:KK, :], w1t.ap()[m][:, KK // 2:KK, :])
                    ups = up.tile([P, TOK], FP32, tag="u")
                    for kk in range(KK):
                        nc.tensor.matmul(
                            ups[:], lhsT=w1_sl[:, kk, :], rhs=h2tok[:, kk, :],
                            start=(kk == 0), stop=(kk == KK - 1))
                    nc.scalar.activation(uT[:, m, :], ups[:], AF.Relu,
                                         bias=b1_sb[:, m:m + 1], scale=1.0)
            with tc.tile_pool(name="zp", bufs=4, space="PSUM") as zp:
                zt = [zp.tile([P, C], FP32, tag="z", name=f"z{mm}")
                      for mm in range(TOK // P)]
                for q in range(F // P):
                    w2_sl = ph4l.tile([P, C], BF16, tag="w2", bufs=6)
                    nc.sync.dma_start(w2_sl[:, 0:512], w2t.ap()[:, q, 0:512])
                    nc.gpsimd.dma_start(w2_sl[:, 512:C], w2t.ap()[:, q, 512:C])
                    for mm in range(TOK // P):
                        for nch in range(C // 512):
                            nc.tensor.matmul(
                                zt[mm][:, nch * 512:(nch + 1) * 512],
                                lhsT=uT[:, q, mm * P:(mm + 1) * P],
                                rhs=w2_sl[:, nch * 512:(nch + 1) * 512],
                                start=(q == 0), stop=False)
                for mm in range(TOK // P):
                    for nch in range(C // 512):
                        # += b2 via ones-row product; closes the psum group
                        nc.tensor.matmul(
                            zt[mm][:, nch * 512:(nch + 1) * 512],
                            lhsT=ones1[:, 0:P],
                            rhs=b2_sb[:, nch * 512:(nch + 1) * 512],
                            start=False, stop=True)
                    o_sb = ph4o.tile([P, C], FP32, tag="o")
                    nc.vector.tensor_add(o_sb[:], zt[mm][:], ytok[:, mm, :])
                    nc.sync.dma_start(out.ap()[mm * P:(mm + 1) * P, :], o_sb[:])

    nc.compile()
    return nc

def prep_inputs(x, Wq, bq, Wk, bk, Wv, bv, Wo, bo, W1, b1, W2, b2,
                gamma1, beta1, gamma2, beta2):
    bf = ml_dtypes.bfloat16
    xf = np.asarray(x, np.float32).reshape(TN, C)
    # softmax rows sum to 1, so the v bias is equivalent to adding
    # concat_h(bv) @ Wo to the attention-projection bias
    bo_eff = (np.asarray(bo, np.float64)
              + np.asarray(bv, np.float64).reshape(C) @ np.asarray(Wo, np.float64)
              ).astype(np.float32)
    in_maps = []
    for i in range(NCORE):
        ci = slice(P * i, P * (i + 1))
        hA, hB = 2 * i, 2 * i + 1

        def tile_km(wcat):  # [C, 128] -> [p, kk, m]
            return np.ascontiguousarray(
                wcat.reshape(KK, P, P).transpose(1, 0, 2)).astype(bf)

        wq_cat = np.concatenate([Wq[hA], Wq[hB]], axis=1)
        wk_cat = np.concatenate([Wk[hA], Wk[hB]], axis=1)
        wv_cat = np.concatenate([Wv[hA], Wv[hB]], axis=1)
        in_maps.append({
            "x_c": np.ascontiguousarray(xf[:, ci]),
            "wq": tile_km(wq_cat),
            "wk": tile_km(wk_cat),
            "wv": tile_km(wv_cat),
            "woc": tile_km(np.ascontiguousarray(Wo[:, ci])),
            "w1t": np.ascontiguousarray(
                W1.reshape(KK, P, F // P, P).transpose(2, 1, 0, 3)).astype(bf),
            "w2t": np.ascontiguousarray(
                W2.reshape(F // P, P, C).transpose(1, 0, 2)).astype(bf),
            "bqc": np.concatenate([bq[hA], bq[hB]])[:, None].astype(np.float32),
            "bkc": np.concatenate([bk[hA], bk[hB]])[:, None].astype(np.float32),
            "boc": bo_eff[ci][:, None].astype(np.float32),
            "b1t": np.ascontiguousarray(
                b1.reshape(F // P, P).T).astype(np.float32),
            "b2r": b2[None, :].astype(np.float32),
            "g1": gamma1[ci][:, None].astype(np.float32),
            "be1": beta1[ci][:, None].astype(np.float32),
            "g2": gamma2[ci][:, None].astype(np.float32),
            "be2": beta2[ci][:, None].astype(np.float32),
        })
    return in_maps


def kernel(**inputs):
    inputs = {k: np.asarray(v) for k, v in inputs.items()}
    if "nc" not in _cache:
        _cache["nc"] = build()
    nc = _cache["nc"]
    in_maps = prep_inputs(**inputs)
    res = bass_utils.run_bass_kernel_spmd(nc, in_maps, core_ids=list(range(NCORE)))
    out = np.concatenate([res.results[i]["out"] for i in range(NCORE)], axis=0)
    return out.reshape(B, T, C).astype(np.float32)



# revision 8
# speedup vs baseline: 1.2619x; 1.2619x over previous
"""Trainium2 Bass kernel for nn_Encoder (pre-norm transformer block, LN over
sequence axis) distributed over 8 NeuronCores.

v2 design (Megatron-TP, replicated x):
  - x replicated to every core in bf16 (plus own fp32 channel slice for the
    residual/stats); each core transposes the FULL x^T and applies LN1 with
    stats shared via a tiny [128,4] AllGather -> full h^T local, no big
    activation AllGather.
  - attention head-sharded (2 heads x 2 batches per core), scores computed
    transposed (S^T = k q^T), softmax denom via ones-column in V; exp runs on
    the Scalar engine in [128,1024] tiles (the attention-phase bottleneck);
    denominator reciprocal via PE broadcast + reciprocal_approx_fast on 64
    partitions.
  - Wo Megatron row-parallel: local partial y^T over all C from own heads,
    bf16 ReduceScatter(add) -> own channel slice; LN2 local.
  - FFN Megatron: W1 column-shard / W2 row-shard (1MB weights each, fully
    resident), AllGather(h2^T) in, bf16 ReduceScatter(z partials) out.
  - independent PE work (batch-1 transposes/QKV, Wo(b0)) interleaved into
    attention's exp-bound bubbles to keep the PE p-state high.
  - output channel-sharded [128, 4096]; host assembles + transposes.
"""

import numpy as np
import ml_dtypes
from contextlib import ExitStack

from concourse import bacc, bass_utils
import concourse.bass as bass
import concourse.tile as tile
import concourse.mybir as mybir
from concourse.masks import make_identity

FP32 = mybir.dt.float32
BF16 = mybir.dt.bfloat16
AF = mybir.ActivationFunctionType
ALU = mybir.AluOpType
AX = mybir.AxisListType

B, T, C, H, HS = 2, 2048, 1024, 16, 64
NCORE, P = 8, 128
TN = B * T            # 4096 flat tokens
F = 4 * C             # 4096
FL = F // NCORE       # 512 own FFN dims
MT = FL // P          # 4  own-f m-tiles
CM = C // P           # 8  chan m-tiles
KK = C // P           # 8  k-tiles over C
EPS = 1e-5
RG = [list(range(NCORE))]

_cache = {}


def _ln_stats(nc, pool, xsrc, g_sb, be_sb, A_out, B_out, n):
    """LN coefficients over the free axis of xsrc [P, n] into A_out/B_out
    ([P,1] APs): h = x*A + B. Unbiased var, eps outside sqrt."""
    s1 = pool.tile([P, 1], FP32, tag="s1")
    s2 = pool.tile([P, 1], FP32, tag="s2")
    nc.vector.reduce_sum(s1[:], xsrc, axis=AX.X)
    s2a = pool.tile([P, 1], FP32, tag="s2a")
    for ch in range(2):
        scr = pool.tile([P, n // 2], FP32, tag="scr")
        half = xsrc.rearrange("p (c n) -> p c n", c=2)[:, ch, :]
        nc.vector.scalar_tensor_tensor(
            out=scr[:], in0=half, scalar=1.0, in1=half,
            op0=ALU.mult, op1=ALU.mult,
            accum_out=(s2a[:] if ch == 0 else s2[:]))
    nc.vector.tensor_add(s2[:], s2[:], s2a[:])
    mean = pool.tile([P, 1], FP32, tag="mean")
    nc.vector.tensor_scalar_mul(mean[:], s1[:], 1.0 / n)
    ss = pool.tile([P, 1], FP32, tag="ss")
    nc.vector.tensor_mul(ss[:], s1[:], s1[:])
    var = pool.tile([P, 1], FP32, tag="var")
    nc.vector.scalar_tensor_tensor(
        out=var[:], in0=ss[:], scalar=-1.0 / n, in1=s2[:],
        op0=ALU.mult, op1=ALU.add)
    nc.vector.tensor_scalar_mul(var[:], var[:], 1.0 / (n - 1))
    den = pool.tile([P, 1], FP32, tag="den")
    nc.scalar.sqrt(den[:], var[:])
    nc.vector.tensor_scalar_add(den[:], den[:], EPS)
    rden = pool.tile([P, 1], FP32, tag="rden")
    nc.vector.reciprocal(rden[:], den[:])
    nc.vector.tensor_mul(A_out, g_sb, rden[:])
    mA = pool.tile([P, 1], FP32, tag="mA")
    nc.vector.tensor_scalar_mul(mA[:], mean[:], A_out)
    nc.vector.tensor_sub(B_out, be_sb, mA[:])


def build():
    nc = bacc.Bacc("TRN2", target_bir_lowering=False, debug=False,
                   num_devices=NCORE)

    def EIN(name, shape, dtype):
        return nc.dram_tensor(name, shape, dtype, kind="ExternalInput")

    x_bf = EIN("x_bf", [TN, C], BF16)      # full x, replicated
    x_c = EIN("x_c", [TN, P], FP32)        # own channel slice
    wq = EIN("wq", [P, KK, P], BF16)       # own 2 heads' Wq, kk-tiled
    wk = EIN("wk", [P, KK, P], BF16)
    wv = EIN("wv", [P, KK, P], BF16)
    wor = EIN("wor", [P, CM, P], BF16)     # Wo[own 128 rows,:] -> [p, m, mc]
    w1c = EIN("w1c", [P, KK, FL], BF16)    # W1[:, own cols] kk-tiled
    w2c = EIN("w2c", [P, MT, C], BF16)     # W2[own rows, :] q-tiled
    bqc = EIN("bqc", [P, 1], FP32)
    bkc = EIN("bkc", [P, 1], FP32)
    boc = EIN("boc", [P, 1], FP32)         # bo_eff own chans (post-reduce)
    b1c = EIN("b1c", [P, MT], FP32)
    b2c = EIN("b2c", [P, 1], FP32)
    g1 = EIN("g1", [P, 1], FP32)
    be1 = EIN("be1", [P, 1], FP32)
    g2 = EIN("g2", [P, 1], FP32)
    be2 = EIN("be2", [P, 1], FP32)
    out = nc.dram_tensor("out", [P, TN], FP32, kind="ExternalOutput")

    with tile.TileContext(nc) as tc, ExitStack() as ctx:
        const = ctx.enter_context(tc.tile_pool(name="const", bufs=1))
        dram = ctx.enter_context(tc.tile_pool(name="dram", bufs=1, space="DRAM"))
        persist = ctx.enter_context(tc.tile_pool(name="acts", bufs=1))
        stats = ctx.enter_context(tc.tile_pool(name="stats", bufs=2))
        # PSUM: wA 2x[128,1024]f32 (4 banks) + wS [64,512] (1) + att [65,1024]
        # (2) + tpp [128,1024]bf16 (1) = 8 banks
        wA = ctx.enter_context(tc.tile_pool(name="wA", bufs=2, space="PSUM"))
        wS = ctx.enter_context(tc.tile_pool(name="wS", bufs=1, space="PSUM"))

        idf = const.tile([P, P], FP32)
        make_identity(nc, idf)
        idb = const.tile([P, P], BF16)
        make_identity(nc, idb)
        ones_b = const.tile([1, P], BF16)
        nc.vector.memset(ones_b[:], 1.0)

        def ldconst(pool, t, shape, dt=FP32, eng=None):
            s = pool.tile(shape, dt, name=t.name + "_sb")
            (eng or nc.gpsimd).dma_start(s[:], t.ap())
            return s

        # weights + biases resident (w1/w2 loaded later, in the FFN scope)
        wq_sb = ldconst(const, wq, [P, KK, P], BF16)
        wk_sb = ldconst(const, wk, [P, KK, P], BF16)
        wv_sb = ldconst(const, wv, [P, KK, P], BF16)
        wor_sb = ldconst(const, wor, [P, CM, P], BF16)
        bq_sb = ldconst(const, bqc, [P, 1])
        bk_sb = ldconst(const, bkc, [P, 1])
        bo_sb = ldconst(const, boc, [P, 1])
        b1_sb = ldconst(const, b1c, [P, MT])
        b2_sb = ldconst(const, b2c, [P, 1])
        g1_sb = ldconst(const, g1, [P, 1])
        be1_sb = ldconst(const, be1, [P, 1])
        g2_sb = ldconst(const, g2, [P, 1])
        be2_sb = ldconst(const, be2, [P, 1])

        # persistent activations
        xT = persist.tile([P, B, T], FP32)        # own chans, transposed
        yT = persist.tile([P, B, T], FP32)
        h2T = persist.tile([P, B, T], BF16)
        st_sb = persist.tile([P, 2 * B], FP32)    # own A/B for b0,b1
        ag_sb = persist.tile([P, KK, 2 * B], FP32)  # gathered stats

        # DRAM comm tiles
        st_in = dram.tile([P, 2 * B], FP32, name="st_in")
        st_out = dram.tile([C, 2 * B], FP32, addr_space="Shared", name="st_out")
        rsy_in = [dram.tile([NCORE, P, T], BF16, name=f"rsy_in{b}")
                  for b in range(B)]
        rsy_out = [dram.tile([P, T], BF16, name=f"rsy_out{b}")
                   for b in range(B)]
        h2_in = [dram.tile([P, T], BF16, name=f"h2_in{b}") for b in range(B)]
        h2_out = [dram.tile([C, T], BF16, addr_space="Shared",
                            name=f"h2_out{b}") for b in range(B)]
        rsz_in = [dram.tile([NCORE, P, T], BF16, name=f"rsz_in{b}")
                  for b in range(B)]
        rsz_out = [dram.tile([P, T], BF16, name=f"rsz_out{b}")
                   for b in range(B)]

        # PSUM drains alternate vector / scalar-copy (gpsimd cannot
        # touch PSUM on hardware)
        _rr = [0]

        def zdrain():
            _rr[0] ^= 1
            return nc.vector.tensor_copy if _rr[0] else nc.scalar.copy

        # Wo drain engine: [fn] mutable so b0 (attention window, scalar is
        # exp-bound) uses vector only; b1 alternates vector/scalar
        wo_eng = [nc.vector.tensor_copy]

        # ---- LN2 units shared between attention scope (b0) and FFN (b1) ----
        def ln2_units(b, stg_pool):
            def u1():
                ys = stg_pool.tile([P, T], BF16, tag="ys", name=f"ys{b}")
                nc.sync.dma_start(ys[:], rsy_out[b][:])
                nc.vector.scalar_tensor_tensor(
                    out=yT[:, b, :], in0=ys[:], scalar=bo_sb[:],
                    in1=xT[:, b, :], op0=ALU.add, op1=ALU.add)

            def u2():
                A2 = stats.tile([P, 1], FP32, tag="A2")
                B2 = stats.tile([P, 1], FP32, tag="B2")
                _ln_stats(nc, stats, yT[:, b, :], g2_sb[:], be2_sb[:],
                          A2[:], B2[:], T)
                nc.vector.tensor_scalar(
                    out=h2T[:, b, :], in0=yT[:, b, :],
                    scalar1=A2[:], scalar2=B2[:], op0=ALU.mult, op1=ALU.add)
                nc.sync.dma_start(h2_in[b][:], h2T[:, b, :])
                nc.gpsimd.collective_compute(
                    "AllGather", ALU.bypass, replica_groups=RG,
                    ins=[h2_in[b].opt()], outs=[h2_out[b].opt()])
            return [u1, u2]

        with tc.tile_pool(name="hT", bufs=1) as hTp, \
             tc.tile_pool(name="attp", bufs=1, space="PSUM") as attp, \
             tc.tile_pool(name="psb", bufs=4) as psb, \
             tc.tile_pool(name="qkvp", bufs=1) as qkvp, \
             tc.tile_pool(name="small", bufs=4) as small:

            qT = qkvp.tile([P, B, T], BF16)
            kT = qkvp.tile([P, B, T], BF16)
            vaug = qkvp.tile([P, B, T // P, 130], BF16)
            attnT = qkvp.tile([P, B, T], BF16)
            nc.vector.memset(vaug[:, :, :, 64], 1.0)
            nc.vector.memset(vaug[:, :, :, 129], 1.0)

            hTt = {}

            def hT_of(b):
                if b not in hTt:
                    hTt[b] = hTp.tile([P, KK, T], BF16, tag="hT",
                                      name=f"hT{b}")
                return hTt[b]

            # ---- QKV units ----
            def qk_unit(b, w_sb, bias_sb, dst, j):
                def u():
                    hT = hT_of(b)
                    ps = wA.tile([P, 1024], FP32, tag="wa", name="qkps")
                    h = ps[:, 0:512]
                    for kk in range(KK):
                        nc.tensor.matmul(
                            h, lhsT=w_sb[:, kk, :],
                            rhs=hT[:, kk, j * 512:(j + 1) * 512],
                            start=(kk == 0), stop=(kk == KK - 1))
                    nc.vector.tensor_scalar_add(
                        dst[:, b, j * 512:(j + 1) * 512], h, bias_sb[:])
                return u

            def v_unit(b, tg):       # tg in 0..3, covers 4 tt
                def u():
                    hT = hT_of(b)
                    ps = wA.tile([P, 1024], FP32, tag="wa", name="vps")
                    for q in range(4):
                        tt = tg * 4 + q
                        for kk in range(KK):
                            nc.tensor.matmul(
                                ps[:, q * P:(q + 1) * P],
                                lhsT=hT[:, kk, tt * P:(tt + 1) * P],
                                rhs=wv_sb[:, kk, :],
                                start=(kk == 0), stop=(kk == KK - 1))
                    dst = vaug[:, b, tg * 4:(tg + 1) * 4, :].rearrange(
                        "p tt (h x) -> p tt h x", h=2)[:, :, :, 0:64]
                    srcv = ps[:, 0:512].rearrange(
                        "p (tt h x) -> p tt h x", tt=4, h=2)
                    nc.vector.tensor_copy(dst, srcv)
                return u

            def qkv_units(b):
                us = []
                for j in range(4):
                    us.append(qk_unit(b, wq_sb, bq_sb, qT, j))
                    us.append(qk_unit(b, wk_sb, bk_sb, kT, j))
                for tg in range(4):
                    us.append(v_unit(b, tg))
                return us

            # ---- Wo partial (own heads, all chans) + RS-y units ----
            wost = {}

            def wo_unit(b, m, stg_pool):
                def u():
                    if m == 0:
                        wost[b] = stg_pool.tile([P, CM, T], BF16,
                                                tag="wos", name=f"wost{b}")
                    for j in range(4):
                        ps = wA.tile([P, 1024], FP32, tag="wa",
                                     name="wops")
                        h = ps[:, 0:512]
                        nc.tensor.matmul(
                            h, lhsT=wor_sb[:, m, :],
                            rhs=attnT[:, b, j * 512:(j + 1) * 512],
                            start=True, stop=True)
                        wo_eng[0](
                            wost[b][:, m, j * 512:(j + 1) * 512], h)
                    nc.sync.dma_start(rsy_in[b][m], wost[b][:, m, :])
                return u

            def rs_y(b):
                nc.gpsimd.collective_compute(
                    "ReduceScatter", ALU.add, replica_groups=RG,
                    ins=[rsy_in[b].opt()], outs=[rsy_out[b].opt()])

            # ---- attention ----
            def attention(b, hd, fillers):
                h64 = slice(hd * 64, (hd + 1) * 64)
                for jh in range(2):
                    att = attp.tile([65, 1024], FP32, tag="att",
                                    name=f"att{b}{hd}{jh}")
                    for k in range(T // P):
                        if fillers and (k % 2 == 0):
                            fillers.pop(0)()
                        sc = wA.tile([P, 1024], FP32, tag="wa", name="sc")
                        for j2 in range(2):
                            nc.tensor.matmul(
                                sc[:, j2 * 512:(j2 + 1) * 512],
                                lhsT=kT[h64, b, k * P:(k + 1) * P],
                                rhs=qT[h64, b,
                                       jh * 1024 + j2 * 512:
                                       jh * 1024 + (j2 + 1) * 512],
                                start=True, stop=True)
                        p = psb.tile([P, 1024], BF16, tag="p")
                        nc.scalar.activation(p[:], sc[:], AF.Exp,
                                             scale=float(HS) ** -0.5)
                        for j2 in range(2):
                            nc.tensor.matmul(
                                att[:, j2 * 512:(j2 + 1) * 512],
                                lhsT=vaug[:, b, k, hd * 65:(hd + 1) * 65],
                                rhs=p[:, j2 * 512:(j2 + 1) * 512],
                                start=(k == 0), stop=(k == T // P - 1))
                    # normalize: denom bcast via PE + approx reciprocal
                    for j2 in range(2):
                        den = small.tile([1, 512], BF16, tag="den")
                        nc.vector.tensor_copy(
                            den[:], att[64:65, j2 * 512:(j2 + 1) * 512])
                        rdp = wS.tile([64, 512], FP32, tag="ws")
                        nc.tensor.matmul(rdp[:], lhsT=ones_b[:, 0:64],
                                         rhs=den[:], start=True, stop=True)
                        rd = small.tile([64, 512], FP32, tag="rd")
                        nc.vector.reciprocal_approx_fast(rd[:], rdp[:])
                        nc.vector.tensor_mul(
                            attnT[h64, b,
                                  jh * 1024 + j2 * 512:
                                  jh * 1024 + (j2 + 1) * 512],
                            att[0:64, j2 * 512:(j2 + 1) * 512], rd[:])

            # =========== emission ===========
            with tc.tile_pool(name="xst", bufs=2) as xstp, \
                 tc.tile_pool(name="tpp", bufs=1, space="PSUM") as tpp:
                # input stages (gpsimd queue: cheap DMA issue)
                xst = {}
                for b in range(B):
                    for hf in range(2):
                        s = xstp.tile([P, 8, C], BF16, tag="xst",
                                      name=f"xst{b}{hf}")
                        nc.gpsimd.dma_start(
                            s[:],
                            x_bf.ap()[b * T + hf * 1024:
                                      b * T + (hf + 1) * 1024, :]
                            .rearrange("(tt p) c -> p tt c", p=P))
                        xst[(b, hf)] = s

                def trans_unit(b, kk, half):
                    def u():
                        hT = hT_of(b)
                        tp = tpp.tile([P, 1024], BF16, tag="tp")
                        for q in range(8):
                            tt = half * 8 + q
                            nc.tensor.transpose(
                                tp[:, q * P:(q + 1) * P],
                                xst[(b, tt // 8)][:, tt % 8,
                                                  kk * P:(kk + 1) * P],
                                idb[:])
                        nc.vector.tensor_scalar(
                            out=hT[:, kk, half * 1024:(half + 1) * 1024],
                            in0=tp[:], scalar1=ag_sb[:, kk, 2 * b:2 * b + 1],
                            scalar2=ag_sb[:, kk, 2 * b + 1:2 * b + 2],
                            op0=ALU.mult, op1=ALU.add)
                    return u

                # phase A: own-slice transposes (fp32) + stats + stats-AG
                with tc.tile_pool(name="xc", bufs=1) as xcp:
                    xc_sb = xcp.tile([P, B, T // P, P], FP32)
                    nc.gpsimd.dma_start(
                        xc_sb[:],
                        x_c.ap().rearrange("(b tt p) c -> p b tt c", p=P, b=B))
                    for b in range(B):
                        for g in range(2):   # 8 transposes per wA tile
                            tp = wA.tile([P, 1024], FP32, tag="wa",
                                         name=f"xtp{b}{g}")
                            for q in range(8):
                                tt = g * 8 + q
                                nc.tensor.transpose(
                                    tp[:, q * P:(q + 1) * P],
                                    xc_sb[:, b, tt, :], idf[:])
                            nc.vector.tensor_copy(
                                xT[:, b, g * 1024:(g + 1) * 1024], tp[:])
                        _ln_stats(nc, stats, xT[:, b, :], g1_sb[:], be1_sb[:],
                                  st_sb[:, 2 * b:2 * b + 1],
                                  st_sb[:, 2 * b + 1:2 * b + 2], T)
                    nc.sync.dma_start(st_in[:], st_sb[:])
                    nc.gpsimd.collective_compute(
                        "AllGather", ALU.bypass, replica_groups=RG,
                        ins=[st_in.opt()], outs=[st_out.opt()])
                    nc.sync.dma_start(
                        ag_sb[:], st_out.rearrange("(kk p) n -> p kk n", p=P))

                # b0 transposes/applies + QKV(b0)
                for kk in range(KK):
                    for half in range(2):
                        trans_unit(0, kk, half)()
                for u in qkv_units(0):
                    u()

                # attention(b0) with b1 prep as fillers
                fill_b0 = []
                for kk in range(KK):
                    for half in range(2):
                        fill_b0.append(trans_unit(1, kk, half))
                fill_b0.extend(qkv_units(1))
                attention(0, 0, fill_b0)
                attention(0, 1, fill_b0)
                while fill_b0:
                    fill_b0.pop(0)()

            with tc.tile_pool(name="stg", bufs=1) as stg:
                # attention(b1): Wo(b0), RS-y(b0), LN2(b0) as fillers
                fill_b1 = [wo_unit(0, m, stg) for m in range(CM)]
                fill_b1.append(lambda: rs_y(0))
                attention(1, 0, fill_b1)
                l2u = ln2_units(0, stg)
                attention(1, 1, fill_b1 + [l2u[0]])
                l2u[1]()

                # Wo(b1) + RS-y(b1): drains alternate vector/scalar
                for m in range(CM):
                    wo_eng[0] = (nc.vector.tensor_copy if m % 2 == 0
                                 else nc.scalar.copy)
                    wo_unit(1, m, stg)()
                wo_eng[0] = nc.vector.tensor_copy
                rs_y(1)

        # =========== FFN (Megatron column/row parallel) ===========
        with tc.tile_pool(name="wffn", bufs=1) as wffn, \
             tc.tile_pool(name="h2f", bufs=1) as h2fp, \
             tc.tile_pool(name="upool", bufs=2) as upool, \
             tc.tile_pool(name="zstg", bufs=1) as zstg, \
             tc.tile_pool(name="fin", bufs=2) as fin:
            w1_sb = ldconst(wffn, w1c, [P, KK, FL], BF16, eng=nc.scalar)
            w2_sb = ldconst(wffn, w2c, [P, MT, C], BF16, eng=nc.scalar)
            h2full = {}

            def h2full_of(b):
                if b not in h2full:
                    h2full[b] = h2fp.tile([P, KK, T], BF16, tag="h2f",
                                          name=f"h2full{b}")
                    nc.sync.dma_start(
                        h2full[b][:],
                        h2_out[b].rearrange("(kk p) n -> p kk n", p=P))
                return h2full[b]

            ln2_b1_done = [False]

            def ffn_batch(b):
                h2f = h2full_of(b)
                zst = zstg.tile([P, CM, T], BF16, tag="zs", name=f"zst{b}")
                for j in range(4):
                    uT = upool.tile([P, MT, 512], BF16, tag="u")
                    for m in range(MT):
                        ps = wA.tile([P, 1024], FP32, tag="wa", name="w1ps")
                        h = ps[:, 0:512]
                        for kk in range(KK):
                            nc.tensor.matmul(
                                h, lhsT=w1_sb[:, kk, m * P:(m + 1) * P],
                                rhs=h2f[:, kk, j * 512:(j + 1) * 512],
                                start=(kk == 0), stop=(kk == KK - 1))
                        nc.scalar.activation(uT[:, m, :], h, AF.Relu,
                                             bias=b1_sb[:, m:m + 1], scale=1.0)
                    for mc in range(CM):
                        ps = wA.tile([P, 1024], FP32, tag="wa", name="w2ps")
                        h = ps[:, 0:512]
                        for q in range(MT):
                            nc.tensor.matmul(
                                h, lhsT=w2_sb[:, q, mc * P:(mc + 1) * P],
                                rhs=uT[:, q, :],
                                start=(q == 0), stop=(q == MT - 1))
                        zdrain()(zst[:, mc, j * 512:(j + 1) * 512], h)
                    # LN2(b1) emitted mid-FFN(b0) so vector reaches it after
                    # RS-y(b1) landed
                    if b == 0 and j == 2 and not ln2_b1_done[0]:
                        ln2_b1_done[0] = True
                        for u in ln2_units(1, zstg):
                            u()
                for mc in range(CM):
                    nc.sync.dma_start(rsz_in[b][mc], zst[:, mc, :])
                nc.gpsimd.collective_compute(
                    "ReduceScatter", ALU.add, replica_groups=RG,
                    ins=[rsz_in[b].opt()], outs=[rsz_out[b].opt()])

            def final(b):
                zs = fin.tile([P, T], BF16, tag="zf", name=f"zf{b}")
                nc.sync.dma_start(zs[:], rsz_out[b][:])
                o = fin.tile([P, T], FP32, tag="o", name=f"o{b}")
                nc.vector.scalar_tensor_tensor(
                    out=o[:], in0=zs[:], scalar=b2_sb[:],
                    in1=yT[:, b, :], op0=ALU.add, op1=ALU.add)
                nc.sync.dma_start(out.ap()[:, b * T:(b + 1) * T], o[:])

            ffn_batch(0)
            ffn_batch(1)
            final(0)
            final(1)

    nc.compile()
    return nc


def prep_inputs(x, Wq, bq, Wk, bk, Wv, bv, Wo, bo, W1, b1, W2, b2,
                gamma1, beta1, gamma2, beta2):
    bf = ml_dtypes.bfloat16
    xf = np.asarray(x, np.float32).reshape(TN, C)
    x_bf_full = np.ascontiguousarray(xf.astype(bf))
    # softmax rows sum to 1, so the v bias is equivalent to adding
    # concat_h(bv) @ Wo to the attention-projection bias
    bo_eff = (np.asarray(bo, np.float64)
              + np.asarray(bv, np.float64).reshape(C) @ np.asarray(Wo, np.float64)
              ).astype(np.float32)
    Wo = np.asarray(Wo, np.float32)
    W1 = np.asarray(W1, np.float32)
    W2 = np.asarray(W2, np.float32)
    in_maps = []
    for i in range(NCORE):
        ci = slice(P * i, P * (i + 1))
        fi = slice(FL * i, FL * (i + 1))
        hA, hB = 2 * i, 2 * i + 1

        def tile_km(wcat):  # [C, 128] -> [p, kk, m]
            return np.ascontiguousarray(
                wcat.reshape(KK, P, P).transpose(1, 0, 2)).astype(bf)

        wq_cat = np.concatenate([Wq[hA], Wq[hB]], axis=1)
        wk_cat = np.concatenate([Wk[hA], Wk[hB]], axis=1)
        wv_cat = np.concatenate([Wv[hA], Wv[hB]], axis=1)
        in_maps.append({
            "x_bf": x_bf_full,
            "x_c": np.ascontiguousarray(xf[:, ci]),
            "wq": tile_km(wq_cat),
            "wk": tile_km(wk_cat),
            "wv": tile_km(wv_cat),
            "wor": np.ascontiguousarray(
                Wo[ci, :].reshape(P, CM, P)).astype(bf),
            "w1c": np.ascontiguousarray(
                W1[:, fi].reshape(KK, P, FL).transpose(1, 0, 2)).astype(bf),
            "w2c": np.ascontiguousarray(
                W2[fi, :].reshape(MT, P, C).transpose(1, 0, 2)).astype(bf),
            "bqc": np.concatenate([bq[hA], bq[hB]])[:, None].astype(np.float32),
            "bkc": np.concatenate([bk[hA], bk[hB]])[:, None].astype(np.float32),
            "boc": bo_eff[ci][:, None].astype(np.float32),
            "b1c": np.ascontiguousarray(
                np.asarray(b1)[fi].reshape(MT, P).T).astype(np.float32),
            "b2c": np.asarray(b2)[ci][:, None].astype(np.float32),
            "g1": np.asarray(gamma1)[ci][:, None].astype(np.float32),
            "be1": np.asarray(beta1)[ci][:, None].astype(np.float32),
            "g2": np.asarray(gamma2)[ci][:, None].astype(np.float32),
            "be2": np.asarray(beta2)[ci][:, None].astype(np.float32),
        })
    return in_maps


def kernel(**inputs):
    inputs = {k: np.asarray(v) for k, v in inputs.items()}
    if "nc" not in _cache:
        _cache["nc"] = build()
    nc = _cache["nc"]
    in_maps = prep_inputs(**inputs)
    res = bass_utils.run_bass_kernel_spmd(nc, in_maps, core_ids=list(range(NCORE)))
    full = np.concatenate([res.results[i]["out"] for i in range(NCORE)], axis=0)
    return np.ascontiguousarray(full.T).reshape(B, T, C).astype(np.float32)
